# revision 20
# baseline (speedup 1.0000x reference)
"""Trainium2 Bass kernel for nn_AttentionBlock (iSQRT-COV attention block).

Pipeline per sample (data-parallel over batch, 16 samples per core):
  1x1 conv (PE, fp32r) -> BN+ReLU (ACT, scale/bias folded) -> center ->
  PE transpose -> Gram matmuls -> trace -> Newton-Schulz sqrtm (5 iters) ->
  fc on symmetrized weights (fp32r) -> sigmoid gate -> x * gate -> out.

All compute stays at partition base 0 (HW quadrant ops at row base 64 and
partition_all_reduce at base 64 are broken on TRN2).
"""

import numpy as np

# Problem constants (hardcoded per contract).
B, C, H, W = 128, 256, 32, 32
HWF = H * W  # 1024
ATT = 64
PLANES = 256
TRI = ATT * (ATT + 1) // 2
BN_EPS = 1e-5
ITER_N = 5
N_CORES = 8
BS = B // N_CORES  # 16 samples per core
HALF = BS // 2  # 8 samples per half-batch

_cache = {}


def _build_module():
    import concourse.bacc as bacc
    import concourse.tile as tile
    import concourse.mybir as mybir
    import concourse.bass_isa as bass_isa

    dt = mybir.dt
    AF = mybir.ActivationFunctionType
    ALU = mybir.AluOpType

    nc = bacc.Bacc("TRN2")

    f32 = dt.float32
    f32r = dt.float32r

    # DRAM I/O (per core). x layout: [sample, c_chunk, c_in_chunk, hw]
    x_d = nc.dram_tensor("x", [BS, 2, 128, HWF], f32r, kind="ExternalInput")
    convw_d = nc.dram_tensor("convw2", [128, 128], f32r, kind="ExternalInput")
    bnscale_d = nc.dram_tensor("bnscale", [64, 1], f32, kind="ExternalInput")
    bnbias_d = nc.dram_tensor("bnbias", [64, 1], f32, kind="ExternalInput")
    ident_d = nc.dram_tensor("ident", [128, 128], f32, kind="ExternalInput")
    c3_d = nc.dram_tensor("c3", [64, 512], f32, kind="ExternalInput")
    eye_d = nc.dram_tensor("eye64", [64, 64], f32, kind="ExternalInput")
    w2t_d = nc.dram_tensor("w2t", [128, 32 * 256], f32r, kind="ExternalInput")
    fcb_d = nc.dram_tensor("fcb8", [8, 256], f32, kind="ExternalInput")
    out_d = nc.dram_tensor("out", [BS, 2, 128, HWF], f32, kind="ExternalOutput")

    with tile.TileContext(nc) as tc:
        import contextlib

        ctx = contextlib.ExitStack()
        with ctx:
            consts = ctx.enter_context(tc.tile_pool(name="consts", bufs=1))
            xpool = ctx.enter_context(tc.tile_pool(name="xpool", bufs=12))
            opool = ctx.enter_context(tc.tile_pool(name="opool", bufs=2))
            ypool = ctx.enter_context(tc.tile_pool(name="ypool", bufs=3))
            ytpool = ctx.enter_context(tc.tile_pool(name="ytpool", bufs=3))
            spool = ctx.enter_context(tc.tile_pool(name="spool", bufs=8))
            nspool = ctx.enter_context(tc.tile_pool(name="nspool", bufs=2))
            gpool = ctx.enter_context(tc.tile_pool(name="gpool", bufs=2))
            # PSUM pools (8 banks total: 2+2+2+2)
            convp = ctx.enter_context(
                tc.tile_pool(name="convp", bufs=2, space="PSUM")
            )
            tpp = ctx.enter_context(tc.tile_pool(name="tpp", bufs=2, space="PSUM"))
            gramp = ctx.enter_context(
                tc.tile_pool(name="gramp", bufs=2, space="PSUM")
            )
            nsp = ctx.enter_context(tc.tile_pool(name="nsp", bufs=2, space="PSUM"))

            # --- load constants ---
            convw = consts.tile([128, 128], f32r)
            nc.sync.dma_start(out=convw, in_=convw_d[:, :])
            bnscale = consts.tile([64, 1], f32)
            nc.sync.dma_start(out=bnscale, in_=bnscale_d[:, :])
            bnbias = consts.tile([64, 1], f32)
            nc.sync.dma_start(out=bnbias, in_=bnbias_d[:, :])
            ident = consts.tile([128, 128], f32)
            nc.sync.dma_start(out=ident, in_=ident_d[:, :])
            c3 = consts.tile([64, 512], f32)
            nc.sync.dma_start(out=c3, in_=c3_d[:, :])
            eye64 = consts.tile([64, 64], f32)
            nc.sync.dma_start(out=eye64, in_=eye_d[:, :])
            w2t = consts.tile([128, 32 * 256], f32r)
            nc.sync.dma_start(out=w2t, in_=w2t_d[:, :])
            fcb8 = consts.tile([8, 256], f32)
            nc.sync.dma_start(out=fcb8, in_=fcb_d[:, :])

            def sample_phase1(s, An8, sfin_all, xts):
                """conv -> bn/relu -> center -> transpose -> gram -> trace.
                Fills An8[:, 64r:64r+64] and sfin_all[:, r] for sample s.
                """
                r = s % HALF

                # load x
                xt = xpool.tile([128, 2 * HWF], f32r, tag="xt")
                nc.sync.dma_start(
                    out=xt[:].rearrange("p (c f) -> p c f", c=2),
                    in_=x_d[s, :, :, :].rearrange("c p f -> p c f"),
                )
                xts.append(xt)

                # conv: out[o, hw] in 2 psum halves of 512
                cps = []
                for nh in range(2):
                    cp = convp.tile([64, 512], f32, tag="convp")
                    for ch in range(2):
                        nc.tensor.matmul(
                            cp[:, :],
                            convw[:, 64 * ch : 64 * ch + 64],
                            xt[
                                :, 1024 * ch + 512 * nh : 1024 * ch + 512 * nh + 512
                            ],
                            start=(ch == 0),
                            stop=(ch == 1),
                        )
                    cps.append(cp)

                # BN + ReLU with row-sum accumulation
                y = ypool.tile([64, HWF], f32, tag="y")
                ms = spool.tile([64, 2], f32, tag="ms")
                for nh in range(2):
                    nc.scalar.activation(
                        out=y[:, 512 * nh : 512 * nh + 512],
                        in_=cps[nh][:, :],
                        func=AF.Relu,
                        bias=bnbias[:, :],
                        scale=bnscale[:, :],
                        accum_out=ms[:, nh : nh + 1],
                    )
                # mneg = -(ms0+ms1)/1024
                mneg = spool.tile([64, 1], f32, tag="mneg")
                nc.vector.tensor_scalar(
                    out=mneg[:, :],
                    in0=ms[:, 0:1],
                    scalar1=ms[:, 1:2],
                    scalar2=-1.0 / HWF,
                    op0=ALU.add,
                    op1=ALU.mult,
                )
                # center y in place
                nc.gpsimd.tensor_scalar_add(
                    out=y[:, :], in0=y[:, :], scalar1=mneg[:, :]
                )

                # transpose y -> yT [128 (hw), 512 (8 chunks x 64)]
                tp = tpp.tile([128, 512], f32, tag="tp")
                for k in range(8):
                    nc.tensor.transpose(
                        tp[:, 64 * k : 64 * k + 64],
                        y[:, 128 * k : 128 * k + 128],
                        ident[0:64, 0:64],
                    )
                yT = ytpool.tile([128, 512], f32, tag="yT")
                nc.scalar.copy(out=yT[:, :], in_=tp[:, :])

                # gram: Gc[i,j] = sum_hw yT[hw,i]*yT[hw,j]
                gc = gramp.tile([64, 64], f32, tag="gc")
                for k in range(8):
                    nc.tensor.matmul(
                        gc[:, :],
                        yT[:, 64 * k : 64 * k + 64],
                        yT[:, 64 * k : 64 * k + 64],
                        start=(k == 0),
                        stop=(k == 7),
                    )
                # H = Gc .* eye ; dcol = diag(Gc)
                hscr = spool.tile([64, 64], f32, tag="hscr")
                dcol = spool.tile([64, 1], f32, tag="dcol")
                nc.vector.tensor_mul(hscr[:, :], gc[:, :], eye64[:, :])
                nc.vector.tensor_reduce(
                    out=dcol[:, :],
                    in_=hscr[:, :],
                    axis=mybir.AxisListType.X,
                    op=ALU.add,
                )
                # TR: all-reduce diag over partitions (broadcast trace)
                trt = spool.tile([64, 1], f32, tag="trt")
                nc.gpsimd.partition_all_reduce(
                    trt[:, :],
                    dcol[:, :],
                    channels=64,
                    reduce_op=bass_isa.ReduceOp.add,
                )
                rcp = spool.tile([64, 1], f32, tag="rcp")
                nc.vector.reciprocal(out=rcp[:, :], in_=trt[:, :])
                # sfin = sqrt(TR/4096)  ( = 0.5*sqrt(normA) )
                nc.scalar.activation(
                    out=sfin_all[:, r : r + 1],
                    in_=trt[:, :],
                    func=AF.Sqrt,
                    scale=1.0 / 4096.0,
                )
                # An = Gc / TR
                nc.vector.tensor_scalar_mul(
                    out=An8[:, 64 * r : 64 * r + 64],
                    in0=gc[:, :],
                    scalar1=rcp[:, :],
                )

            def mm8(lhs8, rhs8, out8):
                """8 per-sample 64x64 matmuls (K=64, base 0) into one bank."""
                for r in range(HALF):
                    g = 64 * r
                    nc.tensor.matmul(
                        out8[:, g : g + 64],
                        lhs8[:, g : g + 64],
                        rhs8[:, g : g + 64],
                        start=True,
                        stop=True,
                    )

            def ns_iterations(An8, sfin_all):
                """Newton-Schulz; returns sq8 [64, 512] sbuf tile."""
                # W0 = 3I - An
                w8 = nspool.tile([64, 512], f32, tag="w8")
                nc.vector.tensor_sub(w8[:, :], c3[:, :], An8[:, :])
                # Z1 = 0.5 W0
                z8 = nspool.tile([64, 512], f32, tag="z8")
                nc.scalar.mul(z8[:, :], w8[:, :], 0.5)
                # Y1 = 0.5 (An @ W0)
                py = nsp.tile([64, 512], f32, tag="nsp")
                mm8(An8, w8, py)
                y8 = nspool.tile([64, 512], f32, tag="y8")
                nc.scalar.mul(y8[:, :], py[:, :], 0.5)

                for _ in range(1, ITER_N - 1):
                    # P = Z @ Y
                    pp = nsp.tile([64, 512], f32, tag="nsp")
                    mm8(z8, y8, pp)
                    wk = nspool.tile([64, 512], f32, tag="w8")
                    nc.vector.tensor_sub(wk[:, :], c3[:, :], pp[:, :])
                    # Ynew = 0.5 Y W ; Znew = 0.5 W Z
                    yp = nsp.tile([64, 512], f32, tag="nsp")
                    mm8(y8, wk, yp)
                    ynew = nspool.tile([64, 512], f32, tag="y8")
                    nc.scalar.mul(ynew[:, :], yp[:, :], 0.5)
                    zp = nsp.tile([64, 512], f32, tag="nsp")
                    mm8(wk, z8, zp)
                    znew = nspool.tile([64, 512], f32, tag="z8")
                    nc.scalar.mul(znew[:, :], zp[:, :], 0.5)
                    y8, z8, w8 = ynew, znew, wk

                # final: R = Y @ (3I - Z Y); sq = R * sfin
                pf = nsp.tile([64, 512], f32, tag="nsp")
                mm8(z8, y8, pf)
                wf = nspool.tile([64, 512], f32, tag="w8")
                nc.vector.tensor_sub(wf[:, :], c3[:, :], pf[:, :])
                rp = nsp.tile([64, 512], f32, tag="nsp")
                mm8(y8, wf, rp)
                sq8 = gpool.tile([64, 512], f32, tag="sq8")
                for r in range(HALF):
                    g = 64 * r
                    nc.vector.tensor_scalar_mul(
                        out=sq8[:, g : g + 64],
                        in0=rp[:, g : g + 64],
                        scalar1=sfin_all[:, r : r + 1],
                    )
                return sq8

            def fc_and_gate(sq8):
                """fc (fp32r) + sigmoid; returns gateT [128, 16] sbuf.

                sqK layout: partition t = 2*i + jhi holds the contiguous
                half-row sq_s[i, 32*jhi : 32*jhi + 32]; free = 32*s + jlo.
                Chunk c (contraction) = all partitions at jlo == c.
                """
                sqK = gpool.tile([128, 256], f32r, tag="sqK")
                for r in range(HALF):
                    src = sq8[:, 64 * r : 64 * r + 64].rearrange(
                        "i (jh jl) -> i jh jl", jh=2
                    )
                    dst = sqK[:, 32 * r : 32 * r + 32]
                    src = src.bitcast(f32r)
                    nc.sync.dma_start(out=dst, in_=src)
                gp = nsp.tile([8, 256], f32, tag="nsp")
                for c in range(32):
                    nc.tensor.matmul(
                        gp[:, :],
                        sqK[:, c : c + 225 : 32],
                        w2t[:, 256 * c : 256 * c + 256],
                        start=(c == 0),
                        stop=(c == 31),
                    )
                gtmp = gpool.tile([8, 256], f32, tag="gtmp")
                nc.vector.tensor_add(gtmp[:, :], gp[:, :], fcb8[:, :])
                gate8 = gpool.tile([8, 256], f32, tag="gate8")
                nc.scalar.activation(
                    out=gate8[:, :], in_=gtmp[:, :], func=AF.Sigmoid
                )
                # transpose gate8 -> gateT [128 (channel), 8*ch + s]
                gtp = gramp.tile([128, 16], f32, tag="gc")
                for ch in range(2):
                    nc.tensor.transpose(
                        gtp[:, 8 * ch : 8 * ch + 8],
                        gate8[:, 128 * ch : 128 * ch + 128],
                        ident[0:8, 0:8],
                    )
                gateT = gpool.tile([128, 16], f32, tag="gateT")
                nc.scalar.copy(out=gateT[:, :], in_=gtp[:, :])
                return gateT

            # ---- main schedule: two half-batches of 8 samples ----
            for h in range(2):
                An8 = nspool.tile([64, 512], f32, tag="an8")
                sfin_all = spool.tile([64, 8], f32, tag="sfin")
                xts = []
                for r in range(HALF):
                    sample_phase1(h * HALF + r, An8, sfin_all, xts)
                sq8 = ns_iterations(An8, sfin_all)
                gateT = fc_and_gate(sq8)
                for r in range(HALF):
                    s = h * HALF + r
                    xt = xts[r]
                    ot = opool.tile([128, 2 * HWF], f32, tag="ot")
                    for ch in range(2):
                        nc.gpsimd.tensor_scalar_mul(
                            out=ot[:, 1024 * ch : 1024 * ch + 1024],
                            in0=xt[:, 1024 * ch : 1024 * ch + 1024].bitcast(f32),
                            scalar1=gateT[:, 8 * ch + r : 8 * ch + r + 1],
                        )
                    nc.sync.dma_start(
                        out=out_d[s, :, :, :].rearrange("c p f -> p c f"),
                        in_=ot[:].rearrange("p (c f) -> p c f", c=2),
                    )

    nc.compile()
    return nc


def _host_consts(conv_w, bn_gamma, bn_beta, bn_mean, bn_var, fc_w, fc_b):
    """Precompute derived constant arrays fed as inputs."""
    conv_w = np.asarray(conv_w, np.float32)
    scale = (np.asarray(bn_gamma, np.float32)
             / np.sqrt(np.asarray(bn_var, np.float32) + BN_EPS)).astype(np.float32)
    bias = (np.asarray(bn_beta, np.float32)
            - np.asarray(bn_mean, np.float32) * scale).astype(np.float32)
    wp = conv_w * scale[:, None]  # [64, 256]
    # convw2[p, 64*ch + o] = wp[o, 128*ch + p]
    convw2 = np.zeros((128, 128), np.float32)
    for ch in range(2):
        convw2[:, 64 * ch : 64 * ch + 64] = wp[:, 128 * ch : 128 * ch + 128].T

    ident = np.eye(128, dtype=np.float32)
    eye64 = np.eye(64, dtype=np.float32)
    c3 = np.tile(3.0 * eye64, (1, 8)).astype(np.float32)  # [64, 512]

    # symmetrized fc weight: W2[q, i, j]
    fc_w = np.asarray(fc_w, np.float32)
    iu = np.triu_indices(ATT)
    M = np.zeros((PLANES, ATT, ATT), np.float32)
    M[:, iu[0], iu[1]] = fc_w
    W2 = (M + M.transpose(0, 2, 1)) * 0.5  # [256, 64, 64]
    # w2t[p, 256*c + q] = W2[q, i=p//2, j=32*(p%2) + c]
    # (matches sqK: partition p = 2*i + jhi, chunk c = jlo)
    w2t = np.ascontiguousarray(
        W2.reshape(PLANES, ATT, 2, 32)  # q, i, jhi, jlo
        .transpose(1, 2, 3, 0)  # i, jhi, jlo, q
        .reshape(128, 32 * 256)
    ).astype(np.float32)

    fcb8 = np.tile(np.asarray(fc_b, np.float32)[None, :], (8, 1)).astype(np.float32)
    return {
        "convw2": convw2,
        "bnscale": scale.reshape(64, 1),
        "bnbias": bias.reshape(64, 1),
        "ident": ident,
        "c3": c3,
        "eye64": eye64,
        "w2t": w2t,
        "fcb8": fcb8,
    }


def _get_module():
    if "nc" not in _cache:
        _cache["nc"] = _build_module()
    return _cache["nc"]


def kernel(x, conv_w, bn_gamma, bn_beta, bn_mean, bn_var, fc_w, fc_b):
    from concourse.bass_utils import run_bass_kernel_spmd

    x = np.asarray(x, np.float32)
    consts = _host_consts(conv_w, bn_gamma, bn_beta, bn_mean, bn_var, fc_w, fc_b)
    nc = _get_module()

    in_maps = []
    for i in range(N_CORES):
        shard = np.ascontiguousarray(
            x[i * BS : (i + 1) * BS].reshape(BS, 2, 128, HWF)
        )
        m = {"x": shard}
        m.update(consts)
        in_maps.append(m)

    res = run_bass_kernel_spmd(nc, in_maps, list(range(N_CORES)))
    _cache["last_result"] = res
    out = np.concatenate(
        [res.results[i]["out"].reshape(BS, C, H, W) for i in range(N_CORES)], axis=0
    )
    return out


# revision 28
# speedup vs baseline: 30.1880x; 30.1880x over previous
"""Trainium2 Bass kernel for nn_AttentionBlock (iSQRT-COV attention block).

Pipeline per sample (data-parallel over batch, 16 samples per core):
  1x1 conv (PE, fp32r) -> BN+ReLU (ACT, scale/bias folded) -> center ->
  PE transpose -> Gram matmuls -> trace -> Newton-Schulz sqrtm (5 iters) ->
  fc on symmetrized weights (fp32r) -> sigmoid gate -> x * gate -> out.

All compute stays at partition base 0 (HW quadrant ops at row base 64 and
partition_all_reduce at base 64 are broken on TRN2).
"""

import numpy as np

# Problem constants (hardcoded per contract).
B, C, H, W = 128, 256, 32, 32
HWF = H * W  # 1024
ATT = 64
PLANES = 256
TRI = ATT * (ATT + 1) // 2
BN_EPS = 1e-5
ITER_N = 5
N_CORES = 8
BS = B // N_CORES  # 16 samples per core
HALF = BS // 2  # 8 samples per half-batch

_cache = {}


def _build_module():
    import concourse.bacc as bacc
    import concourse.tile as tile
    import concourse.mybir as mybir
    import concourse.bass_isa as bass_isa

    dt = mybir.dt
    AF = mybir.ActivationFunctionType
    ALU = mybir.AluOpType

    nc = bacc.Bacc("TRN2")

    f32 = dt.float32
    f32r = dt.float32r
    f16 = dt.float16

    # DRAM I/O (per core). x layout: [sample, c_chunk, c_in_chunk, hw]
    x_d = nc.dram_tensor("x", [BS, 2, 128, HWF], f32r, kind="ExternalInput")
    convw_d = nc.dram_tensor("convw2", [128, 128], f32r, kind="ExternalInput")
    bnscale_d = nc.dram_tensor("bnscale", [64, 1], f32, kind="ExternalInput")
    bnbias_d = nc.dram_tensor("bnbias", [64, 1], f32, kind="ExternalInput")
    ident_d = nc.dram_tensor("ident", [128, 128], f32, kind="ExternalInput")
    c3_d = nc.dram_tensor("c3", [64, 512], f32, kind="ExternalInput")
    eye_d = nc.dram_tensor("eye64", [64, 64], f32, kind="ExternalInput")
    id16_d = nc.dram_tensor("ident16", [64, 64], f16, kind="ExternalInput")
    w2t_d = nc.dram_tensor("w2t", [128, 32 * 256], f32r, kind="ExternalInput")
    fcb_d = nc.dram_tensor("fcb8", [8, 256], f32, kind="ExternalInput")
    out_d = nc.dram_tensor("out", [BS, 2, 128, HWF], f32, kind="ExternalOutput")

    with tile.TileContext(nc) as tc:
        import contextlib

        ctx = contextlib.ExitStack()
        with ctx:
            consts = ctx.enter_context(tc.tile_pool(name="consts", bufs=1))
            xpool = ctx.enter_context(tc.tile_pool(name="xpool", bufs=6))
            opool = ctx.enter_context(tc.tile_pool(name="opool", bufs=2))
            ypool = ctx.enter_context(tc.tile_pool(name="ypool", bufs=3))
            ytpool = ctx.enter_context(tc.tile_pool(name="ytpool", bufs=3))
            spool = ctx.enter_context(tc.tile_pool(name="spool", bufs=8))
            nspool = ctx.enter_context(tc.tile_pool(name="nspool", bufs=2))
            gpool = ctx.enter_context(tc.tile_pool(name="gpool", bufs=2))
            # PSUM pools (8 banks total: 2+2+2+2)
            convp = ctx.enter_context(
                tc.tile_pool(name="convp", bufs=2, space="PSUM")
            )
            tpp = ctx.enter_context(tc.tile_pool(name="tpp", bufs=2, space="PSUM"))
            gramp = ctx.enter_context(
                tc.tile_pool(name="gramp", bufs=2, space="PSUM")
            )
            nsp = ctx.enter_context(tc.tile_pool(name="nsp", bufs=2, space="PSUM"))

            # --- load constants (convw first; bulk via gpsimd queue) ---
            convw = consts.tile([128, 128], f32r)
            nc.sync.dma_start(out=convw, in_=convw_d[:, :])
            bnscale = consts.tile([64, 1], f32)
            nc.gpsimd.dma_start(out=bnscale, in_=bnscale_d[:, :])
            bnbias = consts.tile([64, 1], f32)
            nc.gpsimd.dma_start(out=bnbias, in_=bnbias_d[:, :])
            ident16 = consts.tile([64, 64], f16)
            nc.gpsimd.dma_start(out=ident16, in_=id16_d[:, :])
            ident = consts.tile([128, 128], f32)
            nc.gpsimd.dma_start(out=ident, in_=ident_d[:, :])
            c3 = consts.tile([64, 512], f32)
            nc.gpsimd.dma_start(out=c3, in_=c3_d[:, :])
            eye64 = consts.tile([64, 64], f32)
            nc.gpsimd.dma_start(out=eye64, in_=eye_d[:, :])
            fcb8 = consts.tile([8, 256], f32)
            nc.gpsimd.dma_start(out=fcb8, in_=fcb_d[:, :])
            w2t = consts.tile([128, 32 * 256], f32r)

            def load_w2t():
                # two parallel 2MB chunks, issued late (only fc needs it)
                nc.sync.dma_start(out=w2t[:, 0:4096], in_=w2t_d[:, 0:4096])
                nc.gpsimd.dma_start(out=w2t[:, 4096:8192], in_=w2t_d[:, 4096:8192])

            def load_pair(s, xts, split=False):
                """Load two samples into a [128, 4096] tile."""
                xt2 = xpool.tile([128, 4 * HWF], f32r, tag="xt")
                if split:
                    for j, eng in ((0, nc.sync), (1, nc.gpsimd)):
                        eng.dma_start(
                            out=xt2[:, 2 * HWF * j : 2 * HWF * j + 2 * HWF]
                            .rearrange("p (c f) -> p c f", c=2),
                            in_=x_d[s + j, :, :, :].rearrange("c p f -> p c f"),
                        )
                else:
                    nc.sync.dma_start(
                        out=xt2[:].rearrange("p (s c f) -> p s c f", s=2, c=2),
                        in_=x_d[s : s + 2, :, :, :].rearrange("s c p f -> p s c f"),
                    )
                xts.append(xt2)

            def sample_phase1(s, An8, sfin_all, xts):
                """conv -> bn/relu -> center -> transpose -> gram -> trace.
                Fills An8[:, 64r:64r+64] and sfin_all[:, r] for sample s.
                """
                r = s % HALF
                xt = xts[r // 2][:, 2 * HWF * (r % 2) : 2 * HWF * (r % 2) + 2 * HWF]

                # conv: out[o, hw] in 2 psum halves of 512
                cps = []
                for nh in range(2):
                    cp = convp.tile([64, 512], f32, tag="convp")
                    for ch in range(2):
                        nc.tensor.matmul(
                            cp[:, :],
                            convw[:, 64 * ch : 64 * ch + 64],
                            xt[:, 1024 * ch + 512 * nh : 1024 * ch + 512 * nh + 512],
                            start=(ch == 0),
                            stop=(ch == 1),
                        )
                    cps.append(cp)

                # BN + ReLU with row-sum accumulation
                y = ypool.tile([64, HWF], f16, tag="y")
                ms = spool.tile([64, 2], f32, tag="ms")
                for nh in range(2):
                    nc.scalar.activation(
                        out=y[:, 512 * nh : 512 * nh + 512],
                        in_=cps[nh][:, :],
                        func=AF.Relu,
                        bias=bnbias[:, :],
                        scale=bnscale[:, :],
                        accum_out=ms[:, nh : nh + 1],
                    )
                # mneg = -(ms0+ms1)/1024
                mneg = spool.tile([64, 1], f32, tag="mneg")
                nc.vector.tensor_scalar(
                    out=mneg[:, :],
                    in0=ms[:, 0:1],
                    scalar1=ms[:, 1:2],
                    scalar2=-1.0 / HWF,
                    op0=ALU.add,
                    op1=ALU.mult,
                )
                # center y in place (DVE; gpsimd elementwise is ~30x slower)
                nc.vector.tensor_scalar_add(
                    out=y[:, :], in0=y[:, :], scalar1=mneg[:, :]
                )

                # transpose y -> yT [128 (hw), 512 (8 chunks x 64)]
                tp = tpp.tile([128, 512], f16, tag="tp")
                for k in range(8):
                    nc.tensor.transpose(
                        tp[:, 64 * k : 64 * k + 64],
                        y[:, 128 * k : 128 * k + 128],
                        ident16[:, :],
                    )
                yT = ytpool.tile([128, 512], f16, tag="yT")
                nc.vector.tensor_copy(out=yT[:, :], in_=tp[:, :])

                # gram: Gc[i,j] = sum_hw yT[hw,i]*yT[hw,j]
                gc = gramp.tile([64, 64], f32, tag="gc")
                for k in range(8):
                    nc.tensor.matmul(
                        gc[:, :],
                        yT[:, 64 * k : 64 * k + 64],
                        yT[:, 64 * k : 64 * k + 64],
                        start=(k == 0),
                        stop=(k == 7),
                    )
                # H = Gc .* eye ; dcol = diag(Gc)
                hscr = spool.tile([64, 64], f32, tag="hscr")
                dcol = spool.tile([64, 1], f32, tag="dcol")
                nc.vector.tensor_mul(hscr[:, :], gc[:, :], eye64[:, :])
                nc.vector.tensor_reduce(
                    out=dcol[:, :],
                    in_=hscr[:, :],
                    axis=mybir.AxisListType.X,
                    op=ALU.add,
                )
                # TR: all-reduce diag over partitions (broadcast trace)
                trt = spool.tile([64, 1], f32, tag="trt")
                nc.gpsimd.partition_all_reduce(
                    trt[:, :],
                    dcol[:, :],
                    channels=64,
                    reduce_op=bass_isa.ReduceOp.add,
                )
                rcp = spool.tile([64, 1], f32, tag="rcp")
                nc.vector.reciprocal(out=rcp[:, :], in_=trt[:, :])
                # sfin = sqrt(TR/4096)  ( = 0.5*sqrt(normA) )
                nc.scalar.activation(
                    out=sfin_all[:, r : r + 1],
                    in_=trt[:, :],
                    func=AF.Sqrt,
                    scale=1.0 / 4096.0,
                )
                # An = Gc / TR
                nc.vector.tensor_scalar_mul(
                    out=An8[:, 64 * r : 64 * r + 64],
                    in0=gc[:, :],
                    scalar1=rcp[:, :],
                )

            def mm8(lhs8, rhs8, out8):
                """8 per-sample 64x64 matmuls (K=64, base 0) into one bank."""
                for r in range(HALF):
                    g = 64 * r
                    nc.tensor.matmul(
                        out8[:, g : g + 64],
                        lhs8[:, g : g + 64],
                        rhs8[:, g : g + 64],
                        start=True,
                        stop=True,
                    )

            def ns_iterations(An8, sfin_all):
                """Newton-Schulz; returns sq8 [64, 512] sbuf tile."""
                # W0 = 3I - An
                w8 = nspool.tile([64, 512], f16, tag="w8")
                nc.vector.tensor_sub(w8[:, :], c3[:, :], An8[:, :])
                # Z1 = 0.5 W0
                z8 = nspool.tile([64, 512], f16, tag="z8")
                nc.scalar.mul(z8[:, :], w8[:, :], 0.5)
                # Y1 = 0.5 (An @ W0)
                py = nsp.tile([64, 512], f32, tag="nsp")
                mm8(An8, w8, py)
                y8 = nspool.tile([64, 512], f16, tag="y8")
                nc.scalar.mul(y8[:, :], py[:, :], 0.5)

                for _ in range(1, ITER_N - 1):
                    # P = Z @ Y
                    pp = nsp.tile([64, 512], f32, tag="nsp")
                    mm8(z8, y8, pp)
                    wk = nspool.tile([64, 512], f16, tag="w8")
                    nc.vector.tensor_sub(wk[:, :], c3[:, :], pp[:, :])
                    # Ynew = 0.5 Y W ; Znew = 0.5 W Z
                    yp = nsp.tile([64, 512], f32, tag="nsp")
                    mm8(y8, wk, yp)
                    ynew = nspool.tile([64, 512], f16, tag="y8")
                    nc.scalar.mul(ynew[:, :], yp[:, :], 0.5)
                    zp = nsp.tile([64, 512], f32, tag="nsp")
                    mm8(wk, z8, zp)
                    znew = nspool.tile([64, 512], f16, tag="z8")
                    nc.vector.tensor_scalar_mul(znew[:, :], zp[:, :], 0.5)
                    y8, z8, w8 = ynew, znew, wk

                # final: R = Y @ (3I - Z Y); sq = R * sfin
                pf = nsp.tile([64, 512], f32, tag="nsp")
                mm8(z8, y8, pf)
                wf = nspool.tile([64, 512], f16, tag="w8")
                nc.vector.tensor_sub(wf[:, :], c3[:, :], pf[:, :])
                rp = nsp.tile([64, 512], f32, tag="nsp")
                mm8(y8, wf, rp)
                sq8 = gpool.tile([64, 512], f32, tag="sq8")
                for r in range(HALF):
                    g = 64 * r
                    nc.vector.tensor_scalar_mul(
                        out=sq8[:, g : g + 64],
                        in0=rp[:, g : g + 64],
                        scalar1=sfin_all[:, r : r + 1],
                    )
                return sq8

            def fc_and_gate(sq8):
                """fc (fp32r) + sigmoid; returns gateT [128, 16] sbuf.

                sqK layout: partition t = 2*i + jhi holds the contiguous
                half-row sq_s[i, 32*jhi : 32*jhi + 32]; free = 32*s + jlo.
                Chunk c (contraction) = all partitions at jlo == c.
                """
                sqK = gpool.tile([128, 256], f32r, tag="sqK")
                sq4 = sq8[:].rearrange("i (r two jl) -> i r two jl", two=2, jl=32)
                for jh in range(2):
                    src = sq4[:, :, jh, :]
                    dst = sqK[64 * jh : 64 * jh + 64, :].rearrange(
                        "i (r jl) -> i r jl", jl=32
                    )
                    nc.gpsimd.dma_start(out=dst, in_=src.bitcast(f32r))
                gp = nsp.tile([8, 256], f32, tag="nsp")
                for c in range(32):
                    nc.tensor.matmul(
                        gp[:, :],
                        sqK[:, c : c + 225 : 32],
                        w2t[:, 256 * c : 256 * c + 256],
                        start=(c == 0),
                        stop=(c == 31),
                    )
                gtmp = gpool.tile([8, 256], f32, tag="gtmp")
                nc.vector.tensor_add(gtmp[:, :], gp[:, :], fcb8[:, :])
                gate8 = gpool.tile([8, 256], f32, tag="gate8")
                nc.scalar.activation(
                    out=gate8[:, :], in_=gtmp[:, :], func=AF.Sigmoid
                )
                # transpose gate8 -> gateT [128 (channel), 8*ch + s]
                gtp = gramp.tile([128, 16], f32, tag="gc")
                for ch in range(2):
                    nc.tensor.transpose(
                        gtp[:, 8 * ch : 8 * ch + 8],
                        gate8[:, 128 * ch : 128 * ch + 128],
                        ident[0:8, 0:8],
                    )
                gateT = gpool.tile([128, 16], f32, tag="gateT")
                nc.scalar.copy(out=gateT[:, :], in_=gtp[:, :])
                return gateT

            # ---- main schedule: two half-batches, partially interleaved ----
            def phase1_part(h, part, An8, sfin_all, xts):
                """Emit loads+phase1 for pair-indices [2*part, 2*part+2)."""
                for rp in (2 * part, 2 * part + 1):
                    load_pair(h * HALF + 2 * rp, xts,
                              split=(h == 0 and rp == 0))
                for r in (4 * part, 4 * part + 1, 4 * part + 2, 4 * part + 3):
                    sample_phase1(h * HALF + r, An8, sfin_all, xts)

            def half_state():
                An8 = nspool.tile([64, 512], f16, tag="an8")
                sfin_all = spool.tile([64, 8], f32, tag="sfin")
                return An8, sfin_all, []

            state = {0: half_state(), 1: half_state()}
            a0, s0, x0 = state[0]
            a1, s1, x1 = state[1]
            phase1_part(0, 0, a0, s0, x0)
            load_w2t()
            phase1_part(0, 1, a0, s0, x0)
            for h in range(2):
                An8, sfin_all, xts = state[h]
                sq8 = ns_iterations(An8, sfin_all)
                if h == 0:
                    phase1_part(1, 0, a1, s1, x1)
                gateT = fc_and_gate(sq8)
                for rp in range(HALF // 2):
                    s = h * HALF + 2 * rp
                    xt2 = xts[rp]
                    ot = opool.tile([128, 4 * HWF], f32, tag="ot")
                    for half_s in range(2):
                        r = 2 * rp + half_s
                        off = 2 * HWF * half_s
                        # ch 0 on DVE, ch 1 on ACT to balance engines
                        nc.vector.tensor_scalar_mul(
                            out=ot[:, off : off + 1024],
                            in0=xt2[:, off : off + 1024].bitcast(f32),
                            scalar1=gateT[:, r : r + 1],
                        )
                        nc.scalar.activation(
                            out=ot[:, off + 1024 : off + 2048],
                            in_=xt2[:, off + 1024 : off + 2048].bitcast(f32),
                            func=AF.Copy,
                            scale=gateT[:, 8 + r : 8 + r + 1],
                        )
                    eng = nc.gpsimd if rp % 2 == 0 else nc.sync
                    eng.dma_start(
                        out=out_d[s : s + 2, :, :, :].rearrange(
                            "s c p f -> p s c f"),
                        in_=ot[:].rearrange("p (s c f) -> p s c f", s=2, c=2),
                    )
                if h == 0:
                    phase1_part(1, 1, a1, s1, x1)

    nc.compile()
    return nc


def _host_consts(conv_w, bn_gamma, bn_beta, bn_mean, bn_var, fc_w, fc_b):
    """Precompute derived constant arrays fed as inputs."""
    conv_w = np.asarray(conv_w, np.float32)
    scale = (np.asarray(bn_gamma, np.float32)
             / np.sqrt(np.asarray(bn_var, np.float32) + BN_EPS)).astype(np.float32)
    bias = (np.asarray(bn_beta, np.float32)
            - np.asarray(bn_mean, np.float32) * scale).astype(np.float32)
    wp = conv_w * scale[:, None]  # [64, 256]
    # convw2[p, 64*ch + o] = wp[o, 128*ch + p]
    convw2 = np.zeros((128, 128), np.float32)
    for ch in range(2):
        convw2[:, 64 * ch : 64 * ch + 64] = wp[:, 128 * ch : 128 * ch + 128].T

    ident = np.eye(128, dtype=np.float32)
    eye64 = np.eye(64, dtype=np.float32)
    c3 = np.tile(3.0 * eye64, (1, 8)).astype(np.float32)  # [64, 512]

    # symmetrized fc weight: W2[q, i, j]
    fc_w = np.asarray(fc_w, np.float32)
    iu = np.triu_indices(ATT)
    M = np.zeros((PLANES, ATT, ATT), np.float32)
    M[:, iu[0], iu[1]] = fc_w
    W2 = (M + M.transpose(0, 2, 1)) * 0.5  # [256, 64, 64]
    # w2t[p, 256*c + q] = W2[q, i=p%64, j=32*(p//64) + c]
    # (matches sqK: partition p = 64*jhi + i, chunk c = jlo)
    w2t = np.ascontiguousarray(
        W2.reshape(PLANES, ATT, 2, 32)  # q, i, jhi, jlo
        .transpose(2, 1, 3, 0)  # jhi, i, jlo, q
        .reshape(128, 32 * 256)
    ).astype(np.float32)

    fcb8 = np.tile(np.asarray(fc_b, np.float32)[None, :], (8, 1)).astype(np.float32)
    return {
        "convw2": convw2,
        "bnscale": scale.reshape(64, 1),
        "bnbias": bias.reshape(64, 1),
        "ident": ident,
        "c3": c3,
        "eye64": eye64,
        "ident16": np.eye(64, dtype=np.float16),
        "w2t": w2t,
        "fcb8": fcb8,
    }


def _get_module():
    if "nc" not in _cache:
        _cache["nc"] = _build_module()
    return _cache["nc"]


def kernel(x, conv_w, bn_gamma, bn_beta, bn_mean, bn_var, fc_w, fc_b):
    from concourse.bass_utils import run_bass_kernel_spmd

    x = np.asarray(x, np.float32)
    consts = _host_consts(conv_w, bn_gamma, bn_beta, bn_mean, bn_var, fc_w, fc_b)
    nc = _get_module()

    in_maps = []
    for i in range(N_CORES):
        shard = np.ascontiguousarray(
            x[i * BS : (i + 1) * BS].reshape(BS, 2, 128, HWF)
        )
        m = {"x": shard}
        m.update(consts)
        in_maps.append(m)

    res = run_bass_kernel_spmd(nc, in_maps, list(range(N_CORES)))
    _cache["last_result"] = res
    out = np.concatenate(
        [res.results[i]["out"].reshape(BS, C, H, W) for i in range(N_CORES)], axis=0
    )
    return out


# revision 29
# speedup vs baseline: 30.5435x; 1.0118x over previous
"""Trainium2 Bass kernel for nn_AttentionBlock (iSQRT-COV attention block).

Pipeline per sample (data-parallel over batch, 16 samples per core):
  1x1 conv (PE, fp32r) -> BN+ReLU (ACT, scale/bias folded) -> center ->
  PE transpose -> Gram matmuls -> trace -> Newton-Schulz sqrtm (5 iters) ->
  fc on symmetrized weights (fp32r) -> sigmoid gate -> x * gate -> out.

All compute stays at partition base 0 (HW quadrant ops at row base 64 and
partition_all_reduce at base 64 are broken on TRN2).
"""

import numpy as np

# Problem constants (hardcoded per contract).
B, C, H, W = 128, 256, 32, 32
HWF = H * W  # 1024
ATT = 64
PLANES = 256
TRI = ATT * (ATT + 1) // 2
BN_EPS = 1e-5
ITER_N = 5
N_CORES = 8
BS = B // N_CORES  # 16 samples per core
HALF = BS // 2  # 8 samples per half-batch

_cache = {}


def _build_module():
    import concourse.bacc as bacc
    import concourse.tile as tile
    import concourse.mybir as mybir
    import concourse.bass_isa as bass_isa

    dt = mybir.dt
    AF = mybir.ActivationFunctionType
    ALU = mybir.AluOpType

    nc = bacc.Bacc("TRN2")

    f32 = dt.float32
    f32r = dt.float32r
    f16 = dt.float16

    # DRAM I/O (per core). x layout: [sample, c_chunk, c_in_chunk, hw]
    x_d = nc.dram_tensor("x", [BS, 2, 128, HWF], f32r, kind="ExternalInput")
    convw_d = nc.dram_tensor("convw2", [128, 128], f32r, kind="ExternalInput")
    bnscale_d = nc.dram_tensor("bnscale", [64, 1], f32, kind="ExternalInput")
    bnbias_d = nc.dram_tensor("bnbias", [64, 1], f32, kind="ExternalInput")
    ident_d = nc.dram_tensor("ident", [128, 128], f32, kind="ExternalInput")
    c3_d = nc.dram_tensor("c3", [64, 512], f32, kind="ExternalInput")
    eye_d = nc.dram_tensor("eye64", [64, 64], f32, kind="ExternalInput")
    id16_d = nc.dram_tensor("ident16", [64, 64], f16, kind="ExternalInput")
    w2t_d = nc.dram_tensor("w2t", [128, 32 * 256], f32r, kind="ExternalInput")
    fcb_d = nc.dram_tensor("fcb8", [8, 256], f32, kind="ExternalInput")
    out_d = nc.dram_tensor("out", [BS, 2, 128, HWF], f32, kind="ExternalOutput")

    with tile.TileContext(nc) as tc:
        import contextlib

        ctx = contextlib.ExitStack()
        with ctx:
            consts = ctx.enter_context(tc.tile_pool(name="consts", bufs=1))
            xpool = ctx.enter_context(tc.tile_pool(name="xpool", bufs=6))
            opool = ctx.enter_context(tc.tile_pool(name="opool", bufs=2))
            ypool = ctx.enter_context(tc.tile_pool(name="ypool", bufs=3))
            ytpool = ctx.enter_context(tc.tile_pool(name="ytpool", bufs=3))
            spool = ctx.enter_context(tc.tile_pool(name="spool", bufs=8))
            nspool = ctx.enter_context(tc.tile_pool(name="nspool", bufs=2))
            gpool = ctx.enter_context(tc.tile_pool(name="gpool", bufs=2))
            # PSUM pools (8 banks total: 2+2+2+2)
            convp = ctx.enter_context(
                tc.tile_pool(name="convp", bufs=2, space="PSUM")
            )
            tpp = ctx.enter_context(tc.tile_pool(name="tpp", bufs=2, space="PSUM"))
            gramp = ctx.enter_context(
                tc.tile_pool(name="gramp", bufs=2, space="PSUM")
            )
            nsp = ctx.enter_context(tc.tile_pool(name="nsp", bufs=2, space="PSUM"))

            # --- load constants (convw first; bulk via gpsimd queue) ---
            convw = consts.tile([128, 128], f32r)
            nc.sync.dma_start(out=convw, in_=convw_d[:, :])
            bnscale = consts.tile([64, 1], f32)
            nc.gpsimd.dma_start(out=bnscale, in_=bnscale_d[:, :])
            bnbias = consts.tile([64, 1], f32)
            nc.gpsimd.dma_start(out=bnbias, in_=bnbias_d[:, :])
            ident16 = consts.tile([64, 64], f16)
            nc.gpsimd.dma_start(out=ident16, in_=id16_d[:, :])
            ident = consts.tile([128, 128], f32)
            nc.gpsimd.dma_start(out=ident, in_=ident_d[:, :])
            c3 = consts.tile([64, 512], f32)
            nc.gpsimd.dma_start(out=c3, in_=c3_d[:, :])
            eye64 = consts.tile([64, 64], f32)
            nc.gpsimd.dma_start(out=eye64, in_=eye_d[:, :])
            fcb8 = consts.tile([8, 256], f32)
            nc.gpsimd.dma_start(out=fcb8, in_=fcb_d[:, :])
            w2t = consts.tile([128, 32 * 256], f32r)

            def load_w2t():
                # two parallel 2MB chunks, issued late (only fc needs it)
                nc.sync.dma_start(out=w2t[:, 0:4096], in_=w2t_d[:, 0:4096])
                nc.gpsimd.dma_start(out=w2t[:, 4096:8192], in_=w2t_d[:, 4096:8192])

            def load_pair(s, xts, split=False):
                """Load two samples into a [128, 4096] tile."""
                xt2 = xpool.tile([128, 4 * HWF], f32r, tag="xt")
                if split:
                    for j, eng in ((0, nc.sync), (1, nc.gpsimd)):
                        eng.dma_start(
                            out=xt2[:, 2 * HWF * j : 2 * HWF * j + 2 * HWF]
                            .rearrange("p (c f) -> p c f", c=2),
                            in_=x_d[s + j, :, :, :].rearrange("c p f -> p c f"),
                        )
                else:
                    nc.sync.dma_start(
                        out=xt2[:].rearrange("p (s c f) -> p s c f", s=2, c=2),
                        in_=x_d[s : s + 2, :, :, :].rearrange("s c p f -> p s c f"),
                    )
                xts.append(xt2)

            def sample_phase1(s, An8, sfin_all, xts):
                """conv -> bn/relu -> center -> transpose -> gram -> trace.
                Fills An8[:, 64r:64r+64] and sfin_all[:, r] for sample s.
                """
                r = s % HALF
                xt = xts[r // 2][:, 2 * HWF * (r % 2) : 2 * HWF * (r % 2) + 2 * HWF]

                # conv: out[o, hw] in 2 psum halves of 512
                cps = []
                for nh in range(2):
                    cp = convp.tile([64, 512], f32, tag="convp")
                    for ch in range(2):
                        nc.tensor.matmul(
                            cp[:, :],
                            convw[:, 64 * ch : 64 * ch + 64],
                            xt[:, 1024 * ch + 512 * nh : 1024 * ch + 512 * nh + 512],
                            start=(ch == 0),
                            stop=(ch == 1),
                        )
                    cps.append(cp)

                # BN + ReLU with row-sum accumulation
                y = ypool.tile([64, HWF], f16, tag="y")
                ms = spool.tile([64, 2], f32, tag="ms")
                for nh in range(2):
                    nc.scalar.activation(
                        out=y[:, 512 * nh : 512 * nh + 512],
                        in_=cps[nh][:, :],
                        func=AF.Relu,
                        bias=bnbias[:, :],
                        scale=bnscale[:, :],
                        accum_out=ms[:, nh : nh + 1],
                    )
                # mneg = -(ms0+ms1)/1024
                mneg = spool.tile([64, 1], f32, tag="mneg")
                nc.vector.tensor_scalar(
                    out=mneg[:, :],
                    in0=ms[:, 0:1],
                    scalar1=ms[:, 1:2],
                    scalar2=-1.0 / HWF,
                    op0=ALU.add,
                    op1=ALU.mult,
                )
                # center y in place (DVE; gpsimd elementwise is ~30x slower)
                nc.vector.tensor_scalar_add(
                    out=y[:, :], in0=y[:, :], scalar1=mneg[:, :]
                )

                # transpose y -> yT [128 (hw), 512 (8 chunks x 64)]
                tp = tpp.tile([128, 512], f16, tag="tp")
                for k in range(8):
                    nc.tensor.transpose(
                        tp[:, 64 * k : 64 * k + 64],
                        y[:, 128 * k : 128 * k + 128],
                        ident16[:, :],
                    )
                yT = ytpool.tile([128, 512], f16, tag="yT")
                nc.vector.tensor_copy(out=yT[:, :], in_=tp[:, :])

                # gram: Gc[i,j] = sum_hw yT[hw,i]*yT[hw,j]
                gc = gramp.tile([64, 64], f32, tag="gc")
                for k in range(8):
                    nc.tensor.matmul(
                        gc[:, :],
                        yT[:, 64 * k : 64 * k + 64],
                        yT[:, 64 * k : 64 * k + 64],
                        start=(k == 0),
                        stop=(k == 7),
                    )
                # H = Gc .* eye ; dcol = diag(Gc)
                hscr = spool.tile([64, 64], f32, tag="hscr")
                dcol = spool.tile([64, 1], f32, tag="dcol")
                nc.vector.tensor_mul(hscr[:, :], gc[:, :], eye64[:, :])
                nc.vector.tensor_reduce(
                    out=dcol[:, :],
                    in_=hscr[:, :],
                    axis=mybir.AxisListType.X,
                    op=ALU.add,
                )
                # TR: all-reduce diag over partitions (broadcast trace)
                trt = spool.tile([64, 1], f32, tag="trt")
                nc.gpsimd.partition_all_reduce(
                    trt[:, :],
                    dcol[:, :],
                    channels=64,
                    reduce_op=bass_isa.ReduceOp.add,
                )
                rcp = spool.tile([64, 1], f32, tag="rcp")
                nc.vector.reciprocal(out=rcp[:, :], in_=trt[:, :])
                # sfin = sqrt(TR/4096)  ( = 0.5*sqrt(normA) )
                nc.scalar.activation(
                    out=sfin_all[:, r : r + 1],
                    in_=trt[:, :],
                    func=AF.Sqrt,
                    scale=1.0 / 4096.0,
                )
                # An = Gc / TR
                nc.vector.tensor_scalar_mul(
                    out=An8[:, 64 * r : 64 * r + 64],
                    in0=gc[:, :],
                    scalar1=rcp[:, :],
                )

            def mm8(lhs8, rhs8, out8, nr):
                """nr per-sample 64x64 matmuls (K=64, base 0) into one bank."""
                for r in range(nr):
                    g = 64 * r
                    nc.tensor.matmul(
                        out8[:, g : g + 64],
                        lhs8[:, g : g + 64],
                        rhs8[:, g : g + 64],
                        start=True,
                        stop=True,
                    )

            def ns_iterations(An_ap, sfin_all, r0, nr):
                """Newton-Schulz on samples [r0, r0+nr); returns sq tile."""
                wd = 64 * nr
                # W0 = 3I - An
                w8 = nspool.tile([64, wd], f16, tag="w8")
                nc.vector.tensor_sub(w8[:, :], c3[:, 0:wd], An_ap)
                # Z1 = 0.5 W0
                z8 = nspool.tile([64, wd], f16, tag="z8")
                nc.scalar.mul(z8[:, :], w8[:, :], 0.5)
                # Y1 = 0.5 (An @ W0)
                py = nsp.tile([64, wd], f32, tag="nsp")
                mm8(An_ap, w8, py, nr)
                y8 = nspool.tile([64, wd], f16, tag="y8")
                nc.scalar.mul(y8[:, :], py[:, :], 0.5)

                for _ in range(1, ITER_N - 1):
                    # P = Z @ Y
                    pp = nsp.tile([64, wd], f32, tag="nsp")
                    mm8(z8, y8, pp, nr)
                    wk = nspool.tile([64, wd], f16, tag="w8")
                    nc.vector.tensor_sub(wk[:, :], c3[:, 0:wd], pp[:, :])
                    # Ynew = 0.5 Y W ; Znew = 0.5 W Z
                    yp = nsp.tile([64, wd], f32, tag="nsp")
                    mm8(y8, wk, yp, nr)
                    ynew = nspool.tile([64, wd], f16, tag="y8")
                    nc.scalar.mul(ynew[:, :], yp[:, :], 0.5)
                    zp = nsp.tile([64, wd], f32, tag="nsp")
                    mm8(wk, z8, zp, nr)
                    znew = nspool.tile([64, wd], f16, tag="z8")
                    nc.vector.tensor_scalar_mul(znew[:, :], zp[:, :], 0.5)
                    y8, z8, w8 = ynew, znew, wk

                # final: R = Y @ (3I - Z Y); sq = R * sfin
                pf = nsp.tile([64, wd], f32, tag="nsp")
                mm8(z8, y8, pf, nr)
                wf = nspool.tile([64, wd], f16, tag="w8")
                nc.vector.tensor_sub(wf[:, :], c3[:, 0:wd], pf[:, :])
                rp = nsp.tile([64, wd], f32, tag="nsp")
                mm8(y8, wf, rp, nr)
                sq8 = gpool.tile([64, wd], f32, tag="sq8")
                for r in range(nr):
                    g = 64 * r
                    nc.vector.tensor_scalar_mul(
                        out=sq8[:, g : g + 64],
                        in0=rp[:, g : g + 64],
                        scalar1=sfin_all[:, r0 + r : r0 + r + 1],
                    )
                return sq8

            def fc_and_gate(sq8, nr):
                """fc (fp32r) + sigmoid for nr samples; returns gateT
                [128, 2*nr] sbuf (cols nr*ch + local_r).

                sqK layout: partition t = 64*jhi + i holds the contiguous
                half-row sq_s[i, 32*jhi : 32*jhi + 32]; free = 32*s + jlo.
                Chunk c (contraction) = all partitions at jlo == c.
                """
                wk_ = 32 * nr
                sqK = gpool.tile([128, wk_], f32r, tag="sqK")
                sq4 = sq8[:].rearrange("i (r two jl) -> i r two jl", two=2, jl=32)
                for jh in range(2):
                    src = sq4[:, :, jh, :]
                    dst = sqK[64 * jh : 64 * jh + 64, :].rearrange(
                        "i (r jl) -> i r jl", jl=32
                    )
                    nc.gpsimd.dma_start(out=dst, in_=src.bitcast(f32r))
                gp = nsp.tile([nr, 256], f32, tag="nsp")
                for c in range(32):
                    nc.tensor.matmul(
                        gp[:, :],
                        sqK[:, c : c + 32 * (nr - 1) + 1 : 32],
                        w2t[:, 256 * c : 256 * c + 256],
                        start=(c == 0),
                        stop=(c == 31),
                    )
                gtmp = gpool.tile([nr, 256], f32, tag="gtmp")
                nc.vector.tensor_add(gtmp[:, :], gp[:, :], fcb8[0:nr, :])
                gate8 = gpool.tile([nr, 256], f32, tag="gate8")
                nc.scalar.activation(
                    out=gate8[:, :], in_=gtmp[:, :], func=AF.Sigmoid
                )
                # transpose gate8 -> gateT [128 (channel), nr*ch + local_r]
                gtp = gramp.tile([128, 2 * nr], f32, tag="gc")
                for ch in range(2):
                    nc.tensor.transpose(
                        gtp[:, nr * ch : nr * ch + nr],
                        gate8[:, 128 * ch : 128 * ch + 128],
                        ident[0:nr, 0:nr],
                    )
                gateT = gpool.tile([128, 2 * nr], f32, tag="gateT")
                nc.scalar.copy(out=gateT[:, :], in_=gtp[:, :])
                return gateT

            # ---- main schedule: two half-batches, partially interleaved ----
            def phase1_part(h, part, An8, sfin_all, xts):
                """Emit loads+phase1 for pair-indices [2*part, 2*part+2)."""
                for rp in (2 * part, 2 * part + 1):
                    load_pair(h * HALF + 2 * rp, xts,
                              split=(h == 0 and rp == 0))
                for r in (4 * part, 4 * part + 1, 4 * part + 2, 4 * part + 3):
                    sample_phase1(h * HALF + r, An8, sfin_all, xts)

            def half_state():
                An8 = nspool.tile([64, 512], f16, tag="an8")
                sfin_all = spool.tile([64, 8], f32, tag="sfin")
                return An8, sfin_all, []

            def phase2_pairs(h, xts, gateT, pairs, r0, nr):
                for rp in pairs:
                    s = h * HALF + 2 * rp
                    xt2 = xts[rp]
                    ot = opool.tile([128, 4 * HWF], f32, tag="ot")
                    for half_s in range(2):
                        rr = 2 * rp + half_s - r0
                        off = 2 * HWF * half_s
                        # ch 0 on DVE, ch 1 on ACT to balance engines
                        nc.vector.tensor_scalar_mul(
                            out=ot[:, off : off + 1024],
                            in0=xt2[:, off : off + 1024].bitcast(f32),
                            scalar1=gateT[:, rr : rr + 1],
                        )
                        nc.scalar.activation(
                            out=ot[:, off + 1024 : off + 2048],
                            in_=xt2[:, off + 1024 : off + 2048].bitcast(f32),
                            func=AF.Copy,
                            scale=gateT[:, nr + rr : nr + rr + 1],
                        )
                    eng = nc.gpsimd if rp % 2 == 0 else nc.sync
                    eng.dma_start(
                        out=out_d[s : s + 2, :, :, :].rearrange(
                            "s c p f -> p s c f"),
                        in_=ot[:].rearrange("p (s c f) -> p s c f", s=2, c=2),
                    )

            state = {0: half_state(), 1: half_state()}
            a0, s0, x0 = state[0]
            a1, s1, x1 = state[1]
            phase1_part(0, 0, a0, s0, x0)
            load_w2t()
            phase1_part(0, 1, a0, s0, x0)
            # half 0: full-width NS/fc, interleaved with half-1 phase1
            sq80 = ns_iterations(a0[:, 0:512], s0, 0, 8)
            phase1_part(1, 0, a1, s1, x1)
            gateT0 = fc_and_gate(sq80, 8)
            phase2_pairs(0, x0, gateT0, (0, 1, 2, 3), 0, 8)
            phase1_part(1, 1, a1, s1, x1)
            # half 1: two quarters so the tail overlaps itself
            for q in range(2):
                sqq = ns_iterations(
                    a1[:, 256 * q : 256 * q + 256], s1, 4 * q, 4
                )
                gateTq = fc_and_gate(sqq, 4)
                phase2_pairs(1, x1, gateTq, (2 * q, 2 * q + 1), 4 * q, 4)

    nc.compile()
    return nc


def _host_consts(conv_w, bn_gamma, bn_beta, bn_mean, bn_var, fc_w, fc_b):
    """Precompute derived constant arrays fed as inputs."""
    conv_w = np.asarray(conv_w, np.float32)
    scale = (np.asarray(bn_gamma, np.float32)
             / np.sqrt(np.asarray(bn_var, np.float32) + BN_EPS)).astype(np.float32)
    bias = (np.asarray(bn_beta, np.float32)
            - np.asarray(bn_mean, np.float32) * scale).astype(np.float32)
    wp = conv_w * scale[:, None]  # [64, 256]
    # convw2[p, 64*ch + o] = wp[o, 128*ch + p]
    convw2 = np.zeros((128, 128), np.float32)
    for ch in range(2):
        convw2[:, 64 * ch : 64 * ch + 64] = wp[:, 128 * ch : 128 * ch + 128].T

    ident = np.eye(128, dtype=np.float32)
    eye64 = np.eye(64, dtype=np.float32)
    c3 = np.tile(3.0 * eye64, (1, 8)).astype(np.float32)  # [64, 512]

    # symmetrized fc weight: W2[q, i, j]
    fc_w = np.asarray(fc_w, np.float32)
    iu = np.triu_indices(ATT)
    M = np.zeros((PLANES, ATT, ATT), np.float32)
    M[:, iu[0], iu[1]] = fc_w
    W2 = (M + M.transpose(0, 2, 1)) * 0.5  # [256, 64, 64]
    # w2t[p, 256*c + q] = W2[q, i=p%64, j=32*(p//64) + c]
    # (matches sqK: partition p = 64*jhi + i, chunk c = jlo)
    w2t = np.ascontiguousarray(
        W2.reshape(PLANES, ATT, 2, 32)  # q, i, jhi, jlo
        .transpose(2, 1, 3, 0)  # jhi, i, jlo, q
        .reshape(128, 32 * 256)
    ).astype(np.float32)

    fcb8 = np.tile(np.asarray(fc_b, np.float32)[None, :], (8, 1)).astype(np.float32)
    return {
        "convw2": convw2,
        "bnscale": scale.reshape(64, 1),
        "bnbias": bias.reshape(64, 1),
        "ident": ident,
        "c3": c3,
        "eye64": eye64,
        "ident16": np.eye(64, dtype=np.float16),
        "w2t": w2t,
        "fcb8": fcb8,
    }


def _get_module():
    if "nc" not in _cache:
        _cache["nc"] = _build_module()
    return _cache["nc"]


def kernel(x, conv_w, bn_gamma, bn_beta, bn_mean, bn_var, fc_w, fc_b):
    from concourse.bass_utils import run_bass_kernel_spmd

    x = np.asarray(x, np.float32)
    consts = _host_consts(conv_w, bn_gamma, bn_beta, bn_mean, bn_var, fc_w, fc_b)
    nc = _get_module()

    in_maps = []
    for i in range(N_CORES):
        shard = np.ascontiguousarray(
            x[i * BS : (i + 1) * BS].reshape(BS, 2, 128, HWF)
        )
        m = {"x": shard}
        m.update(consts)
        in_maps.append(m)

    res = run_bass_kernel_spmd(nc, in_maps, list(range(N_CORES)))
    _cache["last_result"] = res
    out = np.concatenate(
        [res.results[i]["out"].reshape(BS, C, H, W) for i in range(N_CORES)], axis=0
    )
    return out
